# revision 1
# baseline (speedup 1.0000x reference)
"""Trainium2 Bass kernel for nn_CriterionCoordinate (pairwise L1-distance
edge loss + masked node loss), data-parallel over segments on 8 cores.

Contract: kernel(**inputs) takes the FULL unsharded inputs
(pred_point [N,3] f32, gt_point [N,3] f32, mask [N,1] f32,
index [n_seg+1] int) and returns the full scalar output (np.float32,
shape ()).
"""

import math
import numpy as np

N_CORES = 8

# Engine assignment / tuning knobs.
CFG = {
    # engine for each of the 6 abs-distance maps: (tensor, coord) ->
    # "act" only for now (abs_max is not a valid DVE/GPSIMD ALU op)
    "abs_eng": {
        (0, 0): "act", (0, 1): "act", (0, 2): "act",
        (1, 0): "act", (1, 1): "act", (1, 2): "act",
    },
    # engine for e_c = ag_c - ap_c combines: "dve" | "gps"
    "e_eng": ["gps", "mix", "dve"],
    "cm_eng": "dve",          # Cm piece pass: "dve" | "gps"
    "s_eng": "dve",           # S piece pass: "dve" | "gps"
    "piece_mode": "stt",      # "stt" (fused 1x) | "tt" (TT+TR at 2x bf16)
    "a_eng": "stt",           # |delta|: "act" | "tr" | "gps" | "stt"
    "map_dtype": "bfloat16",  # dtype of the 6 abs-distance maps
    "mid_dtype": "bfloat16",  # dtype after the e_c cancellation step
    "jb": 1536,               # j-block width for temporaries
    "rb": 1024,               # row-chunk width for partition broadcasts
    "bcast": "dma",           # broadcast build: "gps" | "dma"
    "xj_dtype": "bfloat16",   # dtype of the broadcast coordinate tiles
    "bcast_bufs": 2,          # double-buffer broadcast tiles across segments
    "work_bufs": 3,
}

_PROG_CACHE = {}


def _build_program(spc, P, cfg_key):
    """Build the SPMD Bass program for one core: spc segments of P points."""
    import concourse.bass as bass
    import concourse.tile as tile
    from concourse import bacc, mybir

    cfg = CFG
    f32 = mybir.dt.float32
    mid = mybir.dt.bfloat16 if cfg["mid_dtype"] == "bfloat16" else f32
    mdt = mybir.dt.bfloat16 if cfg["map_dtype"] == "bfloat16" else f32
    xjt = mybir.dt.bfloat16 if cfg["xj_dtype"] == "bfloat16" else f32
    Alu = mybir.AluOpType
    Act = mybir.ActivationFunctionType

    nstr = P // 128           # strips (i-chunks) per segment
    S = spc * nstr            # total i-chunks per core
    JB = min(cfg["jb"], P)
    RB = min(cfg["rb"], 3 * P)

    nc = bacc.Bacc("TRN2", target_bir_lowering=False, debug=False,
                   num_devices=N_CORES)

    xp_pl = nc.dram_tensor("xp_pl", [spc, 3 * P], xjt, kind="ExternalInput").ap()
    xg_pl = nc.dram_tensor("xg_pl", [spc, 3 * P], xjt, kind="ExternalInput").ap()
    m_pl = nc.dram_tensor("m_pl", [spc, P], mid, kind="ExternalInput").ap()
    xi_p = nc.dram_tensor("xi_p", [128, 3 * S], f32, kind="ExternalInput").ap()
    xi_g = nc.dram_tensor("xi_g", [128, 3 * S], f32, kind="ExternalInput").ap()
    nxi_p = nc.dram_tensor("nxi_p", [128, 3 * S], f32, kind="ExternalInput").ap()
    nxi_g = nc.dram_tensor("nxi_g", [128, 3 * S], f32, kind="ExternalInput").ap()
    m_cols = nc.dram_tensor("m_cols", [128, S], f32, kind="ExternalInput").ap()
    mw3 = nc.dram_tensor("mw3", [128, 3 * S], f32, kind="ExternalInput").ap()
    out_d = nc.dram_tensor("out", [128, 8], f32, kind="ExternalOutput").ap()

    with tile.TileContext(nc) as tc:
        with (
            tc.tile_pool(name="const", bufs=1) as cpool,
            tc.tile_pool(name="bcast", bufs=cfg["bcast_bufs"]) as bpool,
            tc.tile_pool(name="rows", bufs=2) as rpool,
            tc.tile_pool(name="work", bufs=cfg["work_bufs"]) as wpool,
            tc.tile_pool(name="junk", bufs=1) as jpool,
        ):
            # ---- constants / small tiles ----
            t_xip = cpool.tile([128, 3 * S], f32, tag="xip")
            t_xig = cpool.tile([128, 3 * S], f32, tag="xig")
            t_nxp = cpool.tile([128, 3 * S], f32, tag="nxp")
            t_nxg = cpool.tile([128, 3 * S], f32, tag="nxg")
            t_mc = cpool.tile([128, S], f32, tag="mc")
            t_mw3 = cpool.tile([128, 3 * S], f32, tag="mw3")
            t_out = cpool.tile([128, 8], f32, tag="outt")
            t_rs = cpool.tile([128, 3 * S], f32, tag="rs")
            t_rc = cpool.tile([128, 3 * S], f32, tag="rc")
            nc.sync.dma_start(out=t_xip[:], in_=xi_p[:])
            nc.sync.dma_start(out=t_xig[:], in_=xi_g[:])
            nc.sync.dma_start(out=t_nxp[:], in_=nxi_p[:])
            nc.sync.dma_start(out=t_nxg[:], in_=nxi_g[:])
            nc.sync.dma_start(out=t_mc[:], in_=m_cols[:])
            nc.sync.dma_start(out=t_mw3[:], in_=mw3[:])
            nc.vector.memset(t_out[:], 0.0)
            nc.vector.memset(t_rs[:], 0.0)
            nc.vector.memset(t_rc[:], 0.0)

            # ---- node loss (tiny) ----
            t_nd3 = jpool.tile([128, 3 * S], f32, tag="nd3")
            t_ndp = jpool.tile([128, S], f32, tag="ndp")
            t_jS = jpool.tile([128, S], f32, tag="jS")
            t_j3S = jpool.tile([128, 3 * S], f32, tag="j3S")
            nc.vector.tensor_sub(t_nd3[:], t_xip[:], t_xig[:])
            nc.vector.tensor_reduce(
                out=t_ndp[:], in_=t_nd3[:].rearrange("p (s c) -> p s c", c=3),
                axis=mybir.AxisListType.X, op=Alu.add,
                apply_absolute_value=True)
            # node_num -> out[:,2]
            nc.vector.scalar_tensor_tensor(
                out=t_jS[:], in0=t_ndp[:], scalar=0.0, in1=t_mc[:],
                op0=Alu.add, op1=Alu.mult, accum_out=t_out[:, 2:3])
            # m_sum -> out[:,3]
            nc.vector.tensor_reduce(out=t_out[:, 3:4], in_=t_mc[:],
                                    axis=mybir.AxisListType.X, op=Alu.add)
            # msq_sum -> out[:,4]
            nc.vector.scalar_tensor_tensor(
                out=t_jS[:], in0=t_mc[:], scalar=0.0, in1=t_mc[:],
                op0=Alu.add, op1=Alu.mult, accum_out=t_out[:, 4:5])

            t_sx = jpool.tile([128, JB], mid, tag="sx")
            t_three = None
            if cfg["piece_mode"] == "tt":
                t_three = cpool.tile([128, JB], mid, tag="three")
                nc.vector.memset(t_three[:], 3.0)

            # ---- main edge-loss loops ----
            for seg in range(spc):
                t_xjp = bpool.tile([128, 3 * P], xjt, tag="xjp")
                t_xjg = bpool.tile([128, 3 * P], xjt, tag="xjg")
                t_mj = bpool.tile([128, P], mid, tag="mj")
                t_mjt = bpool.tile([128, P], mid, tag="mjt")
                if cfg["bcast"] == "dma":
                    for (dst, src, width) in ((t_xjp, xp_pl, 3 * P),
                                              (t_xjg, xg_pl, 3 * P),
                                              (t_mj, m_pl, P)):
                        for k0 in range(0, width, RB):
                            w = min(RB, width - k0)
                            nc.sync.dma_start(
                                out=dst[:, k0:k0 + w],
                                in_=src[seg:seg + 1,
                                        k0:k0 + w].partition_broadcast(128))
                else:
                    for (dst, src) in ((t_xjp, xp_pl), (t_xjg, xg_pl)):
                        for k0 in range(0, 3 * P, RB):
                            w = min(RB, 3 * P - k0)
                            row = rpool.tile([1, RB], xjt, tag="rowx")
                            nc.sync.dma_start(out=row[:1, :w],
                                              in_=src[seg:seg + 1, k0:k0 + w])
                            nc.gpsimd.partition_broadcast(dst[:, k0:k0 + w],
                                                          row[:1, :w])
                    for k0 in range(0, P, RB):
                        w = min(RB, P - k0)
                        rowm = rpool.tile([1, RB], mid, tag="rowm")
                        nc.sync.dma_start(out=rowm[:1, :w],
                                          in_=m_pl[seg:seg + 1, k0:k0 + w])
                        nc.gpsimd.partition_broadcast(t_mj[:, k0:k0 + w],
                                                      rowm[:1, :w])
                # strictly-upper-triangle masked copy of mj per 128-block
                for bi in range(nstr):
                    js = bi * 128
                    nc.gpsimd.affine_select(
                        out=t_mjt[:, js:js + 128], in_=t_mj[:, js:js + 128],
                        pattern=[[1, 128]], compare_op=Alu.is_gt, fill=0.0,
                        base=0, channel_multiplier=-1)

                for bi in range(nstr):
                    chunk = seg * nstr + bi
                    js = bi * 128
                    E = P - js
                    for b in range(math.ceil(E / JB)):
                        jb0 = js + b * JB
                        W = min(JB, P - jb0)
                        # 6 abs-distance maps
                        amaps = {}
                        for t, (xj, xi_t, nxi) in enumerate(
                                ((t_xjg, t_xig, t_nxg), (t_xjp, t_xip, t_nxp))):
                            for c in range(3):
                                m_t = wpool.tile([128, JB], mdt, tag=f"a{t}{c}")
                                src = xj[:, c * P + jb0:c * P + jb0 + W]
                                sc = xi_t[:, 3 * chunk + c:3 * chunk + c + 1]
                                nsc = nxi[:, 3 * chunk + c:3 * chunk + c + 1]
                                eng = cfg["abs_eng"][(t, c)]
                                if eng.startswith("mix_"):
                                    eng = (eng[4:] if (chunk * 4 + b) % 2 == 0
                                           else "act")
                                if eng == "act":
                                    nc.scalar.activation(
                                        m_t[:, :W], src, Act.Abs, bias=nsc,
                                        scale=1.0)
                                else:
                                    # u = xj - xi on gps ("gs") or DVE ("dve2"),
                                    # then |u| = max(-u, u) via one DVE STT
                                    t_u = wpool.tile([128, JB], f32,
                                                     tag=f"u{t}{c}")
                                    ueng = (nc.gpsimd if eng == "gs"
                                            else nc.vector)
                                    ueng.tensor_scalar(
                                        t_u[:, :W], src, sc, None,
                                        Alu.subtract)
                                    nc.vector.scalar_tensor_tensor(
                                        out=m_t[:, :W], in0=t_u[:, :W],
                                        scalar=-1.0, in1=t_u[:, :W],
                                        op0=Alu.mult, op1=Alu.max)
                                amaps[(t, c)] = m_t
                        # e_c = ag_c - ap_c (cancellation step, fp32 in)
                        evs = []
                        blk_i = chunk * 4 + b
                        for c in range(3):
                            e_t = wpool.tile([128, JB], mid, tag=f"e{c}")
                            ecfg = cfg["e_eng"][c]
                            if ecfg == "mix":
                                ecfg = "gps" if blk_i % 2 == 0 else "dve"
                            eng = nc.gpsimd if ecfg == "gps" else nc.vector
                            eng.tensor_sub(e_t[:, :W], amaps[(0, c)][:, :W],
                                           amaps[(1, c)][:, :W])
                            evs.append(e_t)
                        t_e01 = wpool.tile([128, JB], mid, tag="e01")
                        nc.vector.tensor_add(t_e01[:, :W], evs[0][:, :W],
                                             evs[1][:, :W])
                        t_dl = wpool.tile([128, JB], mid, tag="delta")
                        nc.vector.tensor_add(t_dl[:, :W], t_e01[:, :W],
                                             evs[2][:, :W])
                        # aq = |delta|
                        t_aq = wpool.tile([128, JB], mid, tag="aq")
                        if cfg["a_eng"] == "act":
                            nc.scalar.activation(t_aq[:, :W], t_dl[:, :W],
                                                 Act.Abs, bias=0.0, scale=1.0)
                        elif cfg["a_eng"] in ("stt", "gstt"):
                            # |x| = max(-x, x) in one pass
                            a_e = (nc.gpsimd if cfg["a_eng"] == "gstt"
                                   else nc.vector)
                            a_e.scalar_tensor_tensor(
                                out=t_aq[:, :W], in0=t_dl[:, :W], scalar=-1.0,
                                in1=t_dl[:, :W], op0=Alu.mult, op1=Alu.max)
                        elif cfg["a_eng"] == "gps":
                            # |x| = relu(x) - min(x, 0) on gpsimd
                            t_ar = wpool.tile([128, JB], mid, tag="ar")
                            t_an = wpool.tile([128, JB], mid, tag="an")
                            nc.gpsimd.tensor_scalar(t_ar[:, :W], t_dl[:, :W],
                                                    0.0, 0.0, Alu.add, Alu.max)
                            nc.gpsimd.tensor_scalar(t_an[:, :W], t_dl[:, :W],
                                                    0.0, 0.0, Alu.add, Alu.min)
                            nc.gpsimd.tensor_sub(t_aq[:, :W], t_ar[:, :W],
                                                 t_an[:, :W])
                        else:
                            with nc.allow_low_precision("singleton abs-reduce"):
                                nc.vector.tensor_reduce(
                                    out=t_aq[:, :W],
                                    in_=t_dl[:, :W].rearrange(
                                        "p (w o) -> p w o", o=1),
                                    axis=mybir.AxisListType.X, op=Alu.add,
                                    apply_absolute_value=True)
                        # masked count + loss, split diag/rest pieces
                        t_cm = wpool.tile([128, JB], mid, tag="cm")
                        pieces = []
                        if b == 0:
                            pieces.append((0, 128, t_mjt, 0))
                            if W > 128:
                                pieces.append((128, W, t_mj, 1))
                        else:
                            pieces.append((0, W, t_mj, 2))
                        cm_eng = nc.gpsimd if cfg["cm_eng"] == "gps" else nc.vector
                        s_eng = nc.gpsimd if cfg["s_eng"] == "gps" else nc.vector
                        if cfg["piece_mode"] == "tt":
                            t_c = wpool.tile([128, JB], mid, tag="cc")
                            t_s2 = wpool.tile([128, JB], mid, tag="s2")
                            nc.vector.tensor_tensor(
                                t_c[:, :W], t_aq[:, :W], t_three[:, :W],
                                Alu.is_lt)
                            for (o0, o1, wt, slot) in pieces:
                                nc.vector.tensor_tensor(
                                    t_cm[:, o0:o1], t_c[:, o0:o1],
                                    wt[:, jb0 + o0:jb0 + o1], Alu.mult)
                            nc.vector.tensor_tensor(
                                t_s2[:, :W], t_cm[:, :W], t_aq[:, :W],
                                Alu.mult)
                            for (o0, o1, wt, slot) in pieces:
                                col = 3 * chunk + slot
                                nc.vector.tensor_reduce(
                                    out=t_rc[:, col:col + 1],
                                    in_=t_cm[:, o0:o1],
                                    axis=mybir.AxisListType.X, op=Alu.add)
                                nc.vector.tensor_reduce(
                                    out=t_rs[:, col:col + 1],
                                    in_=t_s2[:, o0:o1],
                                    axis=mybir.AxisListType.X, op=Alu.add)
                        else:
                            for (o0, o1, wt, slot) in pieces:
                                col = 3 * chunk + slot
                                cm_eng.scalar_tensor_tensor(
                                    out=t_cm[:, o0:o1], in0=t_aq[:, o0:o1],
                                    scalar=3.0, in1=wt[:, jb0 + o0:jb0 + o1],
                                    op0=Alu.is_lt, op1=Alu.mult,
                                    accum_out=t_rc[:, col:col + 1])
                                s_eng.scalar_tensor_tensor(
                                    out=t_sx[:, o0:o1], in0=t_cm[:, o0:o1],
                                    scalar=0.0, in1=t_aq[:, o0:o1],
                                    op0=Alu.add, op1=Alu.mult,
                                    accum_out=t_rs[:, col:col + 1])

            # ---- final weighted reductions ----
            nc.vector.scalar_tensor_tensor(
                out=t_j3S[:], in0=t_rs[:], scalar=0.0, in1=t_mw3[:],
                op0=Alu.add, op1=Alu.mult, accum_out=t_out[:, 0:1])
            nc.vector.scalar_tensor_tensor(
                out=t_j3S[:], in0=t_rc[:], scalar=0.0, in1=t_mw3[:],
                op0=Alu.add, op1=Alu.mult, accum_out=t_out[:, 1:2])
            nc.sync.dma_start(out=out_d[:], in_=t_out[:])

    nc.compile()
    return nc


def _get_program(spc, P):
    key = (spc, P, str(sorted(CFG.items())))
    if key not in _PROG_CACHE:
        _PROG_CACHE[key] = _build_program(spc, P, key)
    return _PROG_CACHE[key]


def _shard_inputs(pred_point, gt_point, mask, index):
    """Host-side prep: pad + shard segments across cores, build the derived
    small arrays each core needs.

    When the mask is binary (it is, by construction of the problem), points
    with mask==0 contribute nothing to any term, so we compact each segment
    to its masked points (padded to a common multiple of 128, with the
    validity flags taking the role of the mask). This cuts the O(P^2) pair
    work by ~mask_density^2.
    """
    idx = np.asarray(index)
    n_seg = len(idx) - 1
    P = int(idx[1] - idx[0])
    assert np.all(np.diff(idx) == P), "segments must be uniform"
    pred = np.ascontiguousarray(np.asarray(pred_point), dtype=np.float32)
    gt = np.ascontiguousarray(np.asarray(gt_point), dtype=np.float32)
    m = np.ascontiguousarray(np.asarray(mask), dtype=np.float32).reshape(-1)

    binary = bool(np.all((m == 0.0) | (m == 1.0)))
    if binary:
        keep = m.reshape(n_seg, P) == 1.0
        counts = keep.sum(axis=1)
        Pc = max(128, int(math.ceil(counts.max() / 128.0)) * 128)
        predc = np.zeros((n_seg, Pc, 3), np.float32)
        gtc = np.zeros((n_seg, Pc, 3), np.float32)
        mc = np.zeros((n_seg, Pc), np.float32)
        p3 = pred.reshape(n_seg, P, 3)
        g3 = gt.reshape(n_seg, P, 3)
        for s in range(n_seg):
            k = int(counts[s])
            predc[s, :k] = p3[s, keep[s]]
            gtc[s, :k] = g3[s, keep[s]]
            mc[s, :k] = 1.0
        pred = predc.reshape(-1, 3)
        gt = gtc.reshape(-1, 3)
        m = mc.reshape(-1)
        P = Pc
    assert P % 128 == 0, "segment length must be a multiple of 128"

    spc = math.ceil(n_seg / N_CORES)
    n_pad = spc * N_CORES
    if n_pad != n_seg:
        padn = (n_pad - n_seg) * P
        pred = np.concatenate([pred, np.zeros((padn, 3), np.float32)])
        gt = np.concatenate([gt, np.zeros((padn, 3), np.float32)])
        m = np.concatenate([m, np.zeros(padn, np.float32)])

    mid_np = np.dtype(np.float32 if CFG["mid_dtype"] == "float32" else "bfloat16")
    nstr = P // 128
    S = spc * nstr
    Mc = spc * P
    in_maps = []
    for c in range(N_CORES):
        sl = slice(c * Mc, (c + 1) * Mc)
        p_c, g_c, m_c = pred[sl], gt[sl], m[sl]
        xj_np = np.dtype(np.float32 if CFG["xj_dtype"] == "float32"
                         else "bfloat16")
        xp_pl = np.ascontiguousarray(
            p_c.reshape(spc, P, 3).transpose(0, 2, 1)).reshape(
                spc, 3 * P).astype(xj_np)
        xg_pl = np.ascontiguousarray(
            g_c.reshape(spc, P, 3).transpose(0, 2, 1)).reshape(
                spc, 3 * P).astype(xj_np)
        xi_p = np.ascontiguousarray(
            p_c.reshape(S, 128, 3).transpose(1, 0, 2)).reshape(128, 3 * S)
        xi_g = np.ascontiguousarray(
            g_c.reshape(S, 128, 3).transpose(1, 0, 2)).reshape(128, 3 * S)
        m_cols = np.ascontiguousarray(m_c.reshape(S, 128).T)
        mw3 = np.repeat(m_cols, 3, axis=1)
        in_maps.append({
            "xp_pl": xp_pl, "xg_pl": xg_pl,
            "m_pl": m_c.reshape(spc, P).astype(mid_np),
            "xi_p": xi_p, "xi_g": xi_g,
            "nxi_p": -xi_p, "nxi_g": -xi_g,
            "m_cols": m_cols, "mw3": np.ascontiguousarray(mw3),
        })
    return in_maps, spc, P


def _combine(outs):
    """Host-side reduction of per-core [128, 8] partials to the scalar."""
    ps = np.zeros(8, np.float64)
    for o in outs:
        ps += o.astype(np.float64).sum(axis=0)
    edge_loss = 2.0 * ps[0]
    valid = 2.0 * ps[1] + ps[4]
    node = (ps[2] + 1e-9) / (ps[3] + 1e-9)
    if valid >= 1.0:
        res = node + edge_loss / max(valid, 1e-9)
    else:
        res = node
    return np.float32(res)


def kernel(pred_point, gt_point, mask, index):
    from concourse.bass_utils import run_bass_kernel_spmd

    in_maps, spc, P = _shard_inputs(pred_point, gt_point, mask, index)
    nc = _get_program(spc, P)
    res = run_bass_kernel_spmd(nc, in_maps, list(range(N_CORES)))
    return _combine([res.results[c]["out"] for c in range(N_CORES)])



# revision 7
# speedup vs baseline: 1.7060x; 1.7060x over previous
"""Trainium2 Bass kernel for nn_CriterionCoordinate (pairwise L1-distance
edge loss + masked node loss), data-parallel over segments on 8 cores.

Design:
  - Host: compact each segment to its masked points (mask is binary), pad
    with sentinel points whose pairwise |d_gt - d_pred| >> 3 so they are
    auto-excluded by the threshold; node loss computed on host (O(N)).
  - Device (per core, spc segments of P points): for each 128-row i-chunk,
    split into two 64-row halves. For each half a custom DVE op produces
    S = |xj0 - xi0| + |xj1 - xi1| on a stacked tile (partitions 0:64 carry
    the gt plane sums for 64 i-rows, 64:128 the pred plane sums), and the
    Act engine produces M = |xj2 - xi2| stacked the same way. PE matmuls
    with a constant [I64; -I64] weight matrix contract the stacks into
    delta = d_gt - d_pred in PSUM (f32r moving operand, 1 cycle/row).
  - Tail per j-block: either Act Abs (PSUM -> bf16 SBUF) + two 4x-mode
    tensor_scalar passes with accum_out (cnt = sum 1[aq<3], A = sum
    min(aq,3)), or a single custom DVE pass that accumulates
    (aq<3)*(aq+K) (K-packed cnt+loss).  loss = A - 3*W + 3*cnt on host.
  - Diagonal 128x128 blocks are computed full (both orders + self) into
    separate accumulator columns; host halves them and subtracts the 128
    self pairs.
"""

import math
import numpy as np

N_CORES = 8
SENT = 4096.0     # sentinel spacing for padded points
KPACK = 32768.0   # K for the packed custom tail

CFG = {
    "wblk": 512,          # matmul / tail block width (PSUM bank = 512 f32)
    "bcast_bufs": 2,      # per-segment broadcast tile double buffering
    "map_bufs": 3,        # S/M map tile buffering (chunks in flight)
    "psum_bufs": 2,       # PSUM delta tiles in flight (each up to 3 banks)
    "tail": "act",        # "act" (Act abs + 2x TS) | "pack" (custom KPACK)
    "tail_mix": 0,        # if >0: every Nth chunk uses the other tail form
    "xj_dtype": "float32",  # broadcast tile dtype
}

_PROG_CACHE = {}
_OPS = {}


def _get_ops():
    """Register (once) and return the custom DVE ops used by the kernel."""
    if _OPS:
        return _OPS
    from concourse.dve_spec import Spec, Src0, Src1, C0, C1, C2, Zero, maxx, lower
    from concourse.dve_spec import AluOp
    from concourse import dve_ops as dvo
    from concourse.dve_uop import DveOpSpec

    def reg(name, spec):
        for op in dvo.OPS:
            if op.name == name:
                return op
        shas = {}
        for ver in ("v3", "v4"):
            shas[ver] = DveOpSpec(name=name, uops=lower(spec, ver=ver)).sha(ver)
        op = dvo.DveOp(name, spec, subdim=False, uops_sha=shas)
        dvo.OPS.append(op)
        dvo.CUSTOM_DVE_SPECS[name] = spec
        dvo._SUB_OPCODE_FOR_NAME[name] = dvo._CUSTOM_DVE_ROW_BASE + len(dvo.OPS) - 1
        return op

    # S = |in0 - s0| + |in1 - s1|
    abs2s = reg(
        "ABS2S_KNL",
        Spec(
            body=maxx(Src0 - C0, C0 - Src0) + maxx(Src1 - C1, C1 - Src1),
            reference=lambda in0, in1, s0, s1, imm2: (
                np.abs(in0 - s0) + np.abs(in1 - s1)
            ),
        ),
    )

    # packed tail: aq = |in0|; out = (aq < imm2) * (aq + s0);
    # accum_out = s1 + sum(out)
    _aq = maxx(Src0, Zero - Src0)
    tailp = reg(
        "TAILPK_KNL",
        Spec(
            body=(_aq < C2) * (_aq + C0),
            accum=AluOp.ADD,
            accum_init=C1,
            reference=lambda in0, in1, s0, s1, imm2: (
                (np.abs(in0) < imm2) * (np.abs(in0) + s0)
            ),
        ),
    )
    _OPS["abs2s"] = abs2s
    _OPS["tailpk"] = tailp
    return _OPS


def _plan(P, spc, wblk):
    """Block/accumulator-column plan shared by device builder and host
    combine. Returns list of parts (seg, bi, j0, w, kind) where j0 is the
    column offset within the chunk's [i0, P) range; kind 'diag' parts cover
    the full 128x128 diagonal square (host halves them)."""
    nstr = P // 128
    parts = []
    for seg in range(spc):
        for bi in range(nstr):
            E = P - bi * 128
            for b0 in range(0, E, wblk):
                w = min(wblk, E - b0)
                if b0 == 0:
                    parts.append((seg, bi, 0, 128, "diag"))
                    if w > 128:
                        parts.append((seg, bi, 128, w - 128, "rest"))
                else:
                    parts.append((seg, bi, b0, w, "rest"))
    return parts


def _build_program(spc, P, cfg_key):
    import contextlib
    import concourse.bass as bass
    import concourse.tile as tile
    from concourse import bacc, mybir

    cfg = CFG
    f32 = mybir.dt.float32
    f32r = mybir.dt.float32r
    bf16 = mybir.dt.bfloat16
    xjt = f32 if cfg["xj_dtype"] == "float32" else bf16
    mdt = bf16
    Alu = mybir.AluOpType
    Act = mybir.ActivationFunctionType
    ops = _get_ops()

    nstr = P // 128
    WB = cfg["wblk"]
    parts = _plan(P, spc, WB)
    NPART = len(parts)
    # per (seg, chunk) list of matmul blocks [(j0, w)]
    mm_blocks = {}
    for seg in range(spc):
        for bi in range(nstr):
            E = P - bi * 128
            mm_blocks[(seg, bi)] = [
                (b0, min(WB, E - b0)) for b0 in range(0, E, WB)
            ]
    # tail parts grouped per (seg, bi, b0)
    tail_parts = {}
    for pc, (seg, bi, j0, w, kind) in enumerate(parts):
        b0 = 0 if j0 < WB else (j0 // WB) * WB
        tail_parts.setdefault((seg, bi, b0), []).append((j0, w, pc))

    nc = bacc.Bacc("TRN2", target_bir_lowering=False, debug=False,
                   num_devices=N_CORES)

    # DRAM inputs
    bc_d = nc.dram_tensor("bc", [spc, 3, 2, P], xjt, kind="ExternalInput").ap()
    xi_d = nc.dram_tensor("xi", [128, spc * nstr * 2 * 3], f32,
                          kind="ExternalInput").ap()
    nxi_d = nc.dram_tensor("nxi", [128, spc * nstr * 2 * 3], f32,
                           kind="ExternalInput").ap()
    wt_d = nc.dram_tensor("wt", [128, 64], bf16, kind="ExternalInput").ap()
    out_d = nc.dram_tensor("out", [128, 2 * NPART], f32,
                           kind="ExternalOutput").ap()

    def xicol(seg, bi, h, c):
        return ((seg * nstr + bi) * 2 + h) * 3 + c

    with tile.TileContext(nc) as tc:
        with contextlib.ExitStack() as ctx:
            cpool = ctx.enter_context(tc.tile_pool(name="const", bufs=1))
            bpool = ctx.enter_context(
                tc.tile_pool(name="bcast", bufs=cfg["bcast_bufs"]))
            mpool = ctx.enter_context(
                tc.tile_pool(name="maps", bufs=cfg["map_bufs"]))
            ppool = ctx.enter_context(
                tc.tile_pool(name="psum", bufs=cfg["psum_bufs"], space="PSUM"))
            tpool = ctx.enter_context(tc.tile_pool(name="tails", bufs=4))

            t_xi = cpool.tile([128, spc * nstr * 6], f32, tag="xi")
            t_nxi = cpool.tile([128, spc * nstr * 6], f32, tag="nxi")
            t_wt = cpool.tile([128, 64], bf16, tag="wt")
            t_cnt = cpool.tile([128, NPART], f32, tag="cnt")
            t_A = cpool.tile([128, NPART], f32, tag="A")
            nc.sync.dma_start(out=t_xi[:], in_=xi_d[:])
            nc.sync.dma_start(out=t_nxi[:], in_=nxi_d[:])
            nc.sync.dma_start(out=t_wt[:], in_=wt_d[:])
            nc.vector.memset(t_cnt[:], 0.0)
            nc.vector.memset(t_A[:], 0.0)
            wt_r = t_wt[:]

            use_pack = cfg["tail"] == "pack"

            for seg in range(spc):
                # stacked broadcast tiles: partitions 0:64 = gt plane row,
                # 64:128 = pred plane row
                t_b = [bpool.tile([128, P], xjt, name=f"b{c}", tag=f"b{c}")
                       for c in range(3)]
                for c in range(3):
                    for t01, eng in ((0, nc.sync), (1, nc.gpsimd)):
                        eng.dma_start(
                            out=t_b[c][64 * t01:64 * (t01 + 1), :],
                            in_=bc_d[seg, c, t01, :].partition_broadcast(64))

                for bi in range(nstr):
                    i0 = bi * 128
                    E = P - i0
                    # maps for the two 64-row halves
                    t_S = [mpool.tile([128, E], mdt, name=f"S{h}", tag=f"S{h}")
                           for h in range(2)]
                    t_M = [mpool.tile([128, E], mdt, name=f"M{h}", tag=f"M{h}")
                           for h in range(2)]
                    for h in range(2):
                        nc.vector._custom_dve(
                            ops["abs2s"],
                            out=t_S[h][:, :],
                            in0=t_b[0][:, i0:P],
                            in1=t_b[1][:, i0:P],
                            s0=t_xi[:, xicol(seg, bi, h, 0):
                                    xicol(seg, bi, h, 0) + 1],
                            s1=t_xi[:, xicol(seg, bi, h, 1):
                                    xicol(seg, bi, h, 1) + 1],
                        )
                        nc.scalar.activation(
                            t_M[h][:, :], t_b[2][:, i0:P], Act.Abs,
                            bias=t_nxi[:, xicol(seg, bi, h, 2):
                                       xicol(seg, bi, h, 2) + 1],
                            scale=1.0)

                    blocks = mm_blocks[(seg, bi)]
                    for (b0, w) in blocks:
                        t_ps = ppool.tile([128, WB], f32, tag="ps")
                        for h in range(2):
                            po = 64 * h
                            nc.tensor.matmul(
                                t_ps[po:po + 64, :w],
                                lhsT=wt_r[:, :],
                                rhs=t_S[h][:, b0:b0 + w],
                                start=True, stop=False)
                            nc.tensor.matmul(
                                t_ps[po:po + 64, :w],
                                lhsT=wt_r[:, :],
                                rhs=t_M[h][:, b0:b0 + w],
                                start=False, stop=True)
                        tps = tail_parts[(seg, bi, b0)]
                        if use_pack:
                            t_junk = tpool.tile([128, WB], f32, tag="jk")
                            for (j0, w2, pc) in tps:
                                o = j0 - b0
                                nc.vector._custom_dve(
                                    ops["tailpk"],
                                    out=t_junk[:, o:o + w2],
                                    in0=t_ps[:, o:o + w2],
                                    s0=KPACK,
                                    s1=t_cnt[:, pc:pc + 1],
                                    imm2=3.0,
                                    accum_out=t_cnt[:, pc:pc + 1],
                                )
                        else:
                            t_aq = tpool.tile([128, WB], bf16, tag="aq")
                            t_jk = tpool.tile([128, WB], bf16, tag="jk2")
                            nc.scalar.activation(
                                t_aq[:, :w], t_ps[:, :w], Act.Abs,
                                bias=0.0, scale=1.0)
                            for (j0, w2, pc) in tps:
                                o = j0 - b0
                                nc.vector.tensor_scalar(
                                    t_jk[:, o:o + w2], t_aq[:, o:o + w2],
                                    3.0, 0.0, Alu.is_lt, Alu.add,
                                    accum_out=t_cnt[:, pc:pc + 1])
                                nc.vector.tensor_scalar(
                                    t_jk[:, o:o + w2], t_aq[:, o:o + w2],
                                    3.0, 0.0, Alu.min, Alu.add,
                                    accum_out=t_A[:, pc:pc + 1])

            nc.sync.dma_start(out=out_d[:, :NPART], in_=t_cnt[:])
            if not use_pack:
                nc.sync.dma_start(out=out_d[:, NPART:], in_=t_A[:])

    nc.compile()
    return nc


def _get_program(spc, P):
    key = (spc, P, str(sorted(CFG.items())))
    if key not in _PROG_CACHE:
        _PROG_CACHE[key] = _build_program(spc, P, key)
    return _PROG_CACHE[key]


def _shard_inputs(pred_point, gt_point, mask, index):
    """Host-side prep: compact masked points, sentinel-pad, build the
    stacked broadcast rows, per-chunk scalars and weights per core."""
    idx = np.asarray(index)
    n_seg = len(idx) - 1
    P0 = int(idx[1] - idx[0])
    assert np.all(np.diff(idx) == P0), "segments must be uniform"
    pred = np.ascontiguousarray(np.asarray(pred_point), np.float32)
    gt = np.ascontiguousarray(np.asarray(gt_point), np.float32)
    m = np.ascontiguousarray(np.asarray(mask), np.float32).reshape(-1)

    binary = bool(np.all((m == 0.0) | (m == 1.0)))
    keep = (m.reshape(n_seg, P0) == 1.0) if binary else None
    if not binary:
        raise NotImplementedError  # problem spec guarantees binary mask

    counts = keep.sum(axis=1)
    P = max(256, int(math.ceil(counts.max() / 128.0)) * 128)
    p3 = pred.reshape(n_seg, P0, 3)
    g3 = gt.reshape(n_seg, P0, 3)
    predc = np.zeros((n_seg, P, 3), np.float32)
    gtc = np.zeros((n_seg, P, 3), np.float32)
    for s in range(n_seg):
        k = int(counts[s])
        predc[s, :k] = p3[s, keep[s]]
        gtc[s, :k] = g3[s, keep[s]]
        npad = P - k
        if npad:
            predc[s, k:, 0] = SENT * (1.0 + np.arange(npad, dtype=np.float32))

    spc = math.ceil(n_seg / N_CORES)
    n_pad_seg = spc * N_CORES - n_seg
    if n_pad_seg:
        extra_p = np.zeros((n_pad_seg, P, 3), np.float32)
        for s in range(n_pad_seg):
            extra_p[s, :, 0] = SENT * (1.0 + np.arange(P, dtype=np.float32))
        predc = np.concatenate([predc, extra_p])
        gtc = np.concatenate([gtc, np.zeros((n_pad_seg, P, 3), np.float32)])

    nstr = P // 128
    xjt = np.float32 if CFG["xj_dtype"] == "float32" else np.dtype("bfloat16")
    in_maps = []
    for c in range(N_CORES):
        gs = gtc[c * spc:(c + 1) * spc]     # [spc, P, 3]
        ps = predc[c * spc:(c + 1) * spc]
        # bc[s, c, 0, :] = gt plane row; bc[s, c, 1, :] = pred plane row
        bc = np.empty((spc, 3, 2, P), np.float32)
        bc[:, :, 0, :] = gs.transpose(0, 2, 1)
        bc[:, :, 1, :] = ps.transpose(0, 2, 1)
        # xi[128, (seg,bi,h,c)] = concat(gt_c[i0:i0+64], pred_c[i0:i0+64])
        xi = np.empty((128, spc * nstr * 2 * 3), np.float32)
        for s in range(spc):
            for bi in range(nstr):
                for h in range(2):
                    i0 = bi * 128 + 64 * h
                    col = ((s * nstr + bi) * 2 + h) * 3
                    for cc in range(3):
                        xi[:64, col + cc] = gs[s, i0:i0 + 64, cc]
                        xi[64:, col + cc] = ps[s, i0:i0 + 64, cc]
        wt = np.zeros((128, 64), np.float32)
        wt[:64] = np.eye(64, dtype=np.float32)
        wt[64:] = -np.eye(64, dtype=np.float32)
        in_maps.append({
            "bc": bc.astype(xjt),
            "xi": xi,
            "nxi": -xi,
            "wt": wt.astype(np.dtype("bfloat16")),
        })
    return in_maps, spc, P


def _combine(outs, spc, P):
    """Host-side reduction of per-core [128, 2*NPART] partials."""
    parts = _plan(P, spc, CFG["wblk"])
    NPART = len(parts)
    use_pack = CFG["tail"] == "pack"
    tot_cnt = 0.0
    tot_loss = 0.0
    for o in outs:
        o = o.astype(np.float64)
        if use_pack:
            s = o[:, :NPART].sum(axis=0)
            cnt = np.round(s / KPACK)
            loss = s - KPACK * cnt
        else:
            cnt = o[:, :NPART].sum(axis=0)
            A = o[:, NPART:].sum(axis=0)
            w = np.array([128 * p[3] for p in parts], np.float64)
            loss = A - 3.0 * w + 3.0 * cnt
        for i, (seg, bi, j0, wid, kind) in enumerate(parts):
            if kind == "diag":
                tot_cnt += (cnt[i] - 128.0) / 2.0
                tot_loss += loss[i] / 2.0
            else:
                tot_cnt += cnt[i]
                tot_loss += loss[i]
    return tot_loss, tot_cnt


def kernel(pred_point, gt_point, mask, index):
    from concourse.bass_utils import run_bass_kernel_spmd

    pred = np.asarray(pred_point, np.float32)
    gt = np.asarray(gt_point, np.float32)
    m = np.asarray(mask, np.float32).reshape(-1)

    # node loss on host (O(N), negligible)
    diff_coord = np.abs(pred.astype(np.float64)
                        - gt.astype(np.float64)).sum(axis=1)
    node = (float((m * diff_coord).sum()) + 1e-9) / (float(m.sum()) + 1e-9)

    in_maps, spc, P = _shard_inputs(pred_point, gt_point, mask, index)
    nc = _get_program(spc, P)
    res = run_bass_kernel_spmd(nc, in_maps, list(range(N_CORES)))
    edge, valid = _combine([res.results[c]["out"] for c in range(N_CORES)],
                           spc, P)
    if valid >= 1.0:
        out = node + edge / max(valid, 1e-9)
    else:
        out = node
    return np.float32(out)
